# revision 4
# baseline (speedup 1.0000x reference)
"""MoE experts kernel (GPT-OSS style, dense routing over 8 experts) for 8 trn2 NeuronCores.

Strategy: expert-parallel. Core e computes its expert's full MLP for all 4096
tokens, scaled by that expert's routing weight column; the host sums the 8
partial outputs (the unshard step for expert-parallel sharding).

Everything runs in a transposed [feature, token] layout on-chip so that no
transposes are needed anywhere:
  gate   = Wg^T @ X^T          (Wg [H,D] natural = lhsT, X^T [H,T] natural = rhs)
  up     = Wu^T @ X^T
  act    = (up + bu + 1) * gelu_sigmoid(gate + bg)     [D, T] fp16
  out    = (act^T @ Wd + bd) * w_route                 (act tile = lhsT, Wd = rhs)
giving out in [T, H] layout directly. The +bd and *w_route fold into the two
DVE ops that drain the down-projection PSUM.

Matmuls run in fp16 (4x the mantissa of bf16, same PE speed; all values here
are O(10) so fp16 range is ample). PSUM accumulation is fp32.
Measured: ~354 us HW exec per core (PE-roofline ~328 us), rel err 3.4e-4.
"""

import numpy as np

import concourse.mybir as mybir
import concourse.tile as tile
from concourse import bacc
from concourse.bass import ts, ds
from concourse.bass_utils import run_bass_kernel_spmd

AF = mybir.ActivationFunctionType
OP = mybir.AluOpType
F16 = mybir.dt.float16
F32 = mybir.dt.float32

P = 128
H = 1024          # hidden dim
D = 1024          # expert dim
NUM_EXPERTS = 8

# Gelu_apprx_sigmoid LUT computes x*sigmoid(1.702x) in one ACT pass.
# Fallback (False) builds it from Sigmoid + 2 DVE ops (simulator-friendly).
USE_GELU_LUT = True


def build_nc(T=4096, use_gelu_lut=USE_GELU_LUT):
    KT = H // P            # k-tiles for gate/up matmul (contraction over H)
    KD = D // P            # k-tiles for down matmul (contraction over D)
    DT = D // P            # d-tiles of the expert dim
    TCH = 512              # token chunk = psum free dim
    NCH = T // TCH         # token chunks
    TTILES = TCH // P      # 128-token tiles per chunk
    HCH = 512              # h chunk of the down matmul output
    NHCH = H // HCH

    nc = bacc.Bacc("TRN2", debug=False, enable_asserts=False, num_devices=NUM_EXPERTS)

    xt_d = nc.dram_tensor("xt", [H, T], F16, kind="ExternalInput")
    wg_d = nc.dram_tensor("wg", [H, D], F16, kind="ExternalInput")
    wu_d = nc.dram_tensor("wu", [H, D], F16, kind="ExternalInput")
    wd_d = nc.dram_tensor("wd", [D, H], F16, kind="ExternalInput")
    bg_d = nc.dram_tensor("bg", [P, DT], F32, kind="ExternalInput")
    bu1_d = nc.dram_tensor("bu1", [P, DT], F32, kind="ExternalInput")
    bdb_d = nc.dram_tensor("bdb", [P, H], F32, kind="ExternalInput")
    wr_d = nc.dram_tensor("wr", [P, T // P], F32, kind="ExternalInput")
    out_d = nc.dram_tensor("out", [T, H], F32, kind="ExternalOutput")
    out_ap = out_d.ap()

    with tile.TileContext(nc) as tc:
        with (
            tc.tile_pool(name="wpool", bufs=1) as wpool,
            tc.tile_pool(name="xpool", bufs=3) as xpool,
            tc.tile_pool(name="gpool", bufs=3) as gpool,
            tc.tile_pool(name="apool", bufs=2) as apool,
            tc.tile_pool(name="opool", bufs=4) as opool,
            tc.tile_pool(name="pgu", bufs=2, space="PSUM") as pgu,
            tc.tile_pool(name="pdn", bufs=3, space="PSUM") as pdn,
        ):
            # --- resident weights/biases ---
            # DMA *issue* costs ~0.6us each on the sync engine's queue, so
            # the emission order below is chosen to put the first matmul's
            # dependencies at the very front of the queue, and everything
            # else behind the point where it is first consumed.
            bg_sb = wpool.tile([P, DT], F32, name="bg_sb")
            bu1_sb = wpool.tile([P, DT], F32, name="bu1_sb")
            wr_sb = wpool.tile([P, T // P], F32, name="wr_sb")
            bdb_sb = wpool.tile([P, H], F32, name="bdb_sb")

            wg_sb = wpool.tile([P, KT, D], F16, name="wg_sb")
            wu_sb = wpool.tile([P, KT, D], F16, name="wu_sb")
            wd_sb = wpool.tile([P, KD, H], F16, name="wd_sb")
            wg3 = wg_d.ap().rearrange("(ko p) d -> p ko d", p=P)
            wu3 = wu_d.ap().rearrange("(ko p) d -> p ko d", p=P)
            wd3 = wd_d.ap().rearrange("(ko p) h -> p ko h", p=P)

            act_tiles = [None] * NCH
            GLU_BUFS = DT + 2

            def emit_gate_mms(dd, xt_sb):
                pg = pgu.tile([P, TCH], F32, name="pg", bufs=4)
                for k in range(KT):
                    nc.tensor.matmul(
                        pg[:], wg_sb[:, k, ts(dd, P)], xt_sb[:, k, :],
                        start=(k == 0), stop=(k == KT - 1),
                    )
                return pg

            def emit_glu(dd, pg):
                glu_t = gpool.tile([P, TCH], F16, name="glu_t", bufs=GLU_BUFS)
                if use_gelu_lut:
                    # glu = g*sigmoid(1.702 g), g = psum_gate + bg
                    nc.scalar.activation(
                        glu_t[:], pg[:], AF.Gelu_apprx_sigmoid,
                        bias=bg_sb[:, dd:dd + 1], scale=1.0,
                    )
                else:
                    g_t = gpool.tile([P, TCH], F32, name="g_t")
                    nc.vector.tensor_scalar(
                        g_t[:], pg[:], bg_sb[:, dd:dd + 1], None, OP.add,
                    )
                    s_t = gpool.tile([P, TCH], F16, name="s_t")
                    nc.scalar.activation(s_t[:], g_t[:], AF.Sigmoid, scale=1.702)
                    nc.vector.tensor_mul(glu_t[:], g_t[:], s_t[:])
                return glu_t

            def emit_up_act(dd, xt_sb, act_t, glu_t):
                pu = pgu.tile([P, TCH], F32, name="pu", bufs=2)
                for k in range(KT):
                    nc.tensor.matmul(
                        pu[:], wu_sb[:, k, ts(dd, P)], xt_sb[:, k, :],
                        start=(k == 0), stop=(k == KT - 1),
                    )
                # act = (psum_up + (bu+1)) * glu
                nc.vector.scalar_tensor_tensor(
                    act_t[:, dd, :], pu[:], bu1_sb[:, dd:dd + 1], glu_t[:],
                    OP.add, OP.mult,
                )

            def emit_gateup(c, xt_sb):
                act_t = apool.tile([P, DT, TCH], F16, name="act_t")
                act_tiles[c] = act_t
                for dd in range(DT):
                    pg = emit_gate_mms(dd, xt_sb)
                    glu_t = emit_glu(dd, pg)
                    emit_up_act(dd, xt_sb, act_t, glu_t)

            def emit_down(c):
                act_t = act_tiles[c]
                for tt in range(TTILES):
                    tcol = c * TTILES + tt
                    # both h-halves drain into one [P, H] tile -> one out DMA
                    ot = opool.tile([P, H], F32, name="ot", bufs=2)
                    for hh in range(NHCH):
                        po = pdn.tile([P, HCH], F32, name="po", bufs=2)
                        for kd in range(KD):
                            nc.tensor.matmul(
                                po[:], act_t[:, kd, ts(tt, P)], wd_sb[:, kd, ts(hh, HCH)],
                                start=(kd == 0), stop=(kd == KD - 1),
                            )
                        # out = (psum + bd) * w_route[t]
                        qt = opool.tile([P, HCH], F32, name="qt")
                        nc.vector.tensor_add(qt[:], po[:], bdb_sb[:, ts(hh, HCH)])
                        nc.vector.tensor_scalar(
                            ot[:, ts(hh, HCH)], qt[:], wr_sb[:, tcol:tcol + 1], None,
                            OP.mult,
                        )
                    # issue from the (otherwise idle) gpsimd queue: a
                    # different DMA ring than the input loads, so tail
                    # transfers overlap.
                    nc.gpsimd.dma_start(
                        out_ap[ds(c * TCH + tt * P, P), :], ot[:],
                    )

            xt3 = xt_d.ap().rearrange("(ko p) t -> p ko t", p=P)
            for c in range(NCH):
                xt_sb = xpool.tile([P, KT, TCH], F16, name="xt_sb")
                if c == 0:
                    # PE warm-up: matmuls on a zeroed tile, dependent only on
                    # a local memset, stream from ~6.5us (right after the NEFF
                    # prolog) while the first weight DMA is still in flight.
                    # They carry the PE through its p-state ramp so the first
                    # real matmuls run at full clock, and they keep the PE
                    # busy during the first (cold, slow) HBM transfer.
                    warm_sb = wpool.tile([P, TCH], F16, name="warm_sb")
                    nc.gpsimd.memset(warm_sb[:], 0.0)
                    for w in range(8):
                        pwu = pgu.tile([P, TCH], F32, name="pg", bufs=4)
                        nc.tensor.matmul(
                            pwu[:], warm_sb[:, :P], warm_sb[:], start=True, stop=True,
                        )
                    # Startup choreography. The matmul stream becomes dense as
                    # soon as the first k-slices land: the gate phase runs
                    # k-outer over dd-groups of 4 (4 psum banks), so each
                    # arriving (wg_k, xt) slice immediately feeds 4 matmuls.
                    # The first wg slice is split in half (dds 0-3 only) so
                    # the first real matmul's dependency is a 128KB transfer;
                    # xt rides the scalar queue in parallel; biases ride
                    # gpsimd.
                    nc.sync.dma_start(wg_sb[:, 0, :512], wg3[:, 0, :512])
                    nc.scalar.dma_start(xt_sb[:, 0, :], xt3[:, 0, ts(c, TCH)])
                    nc.sync.dma_start(wg_sb[:, 0, 512:], wg3[:, 0, 512:])
                    nc.scalar.dma_start(xt_sb[:, 1:4, :], xt3[:, 1:4, ts(c, TCH)])
                    nc.sync.dma_start(wg_sb[:, 1, :], wg3[:, 1, :])
                    nc.sync.dma_start(wg_sb[:, 2, :], wg3[:, 2, :])
                    nc.sync.dma_start(wg_sb[:, 3, :], wg3[:, 3, :])
                    nc.scalar.dma_start(xt_sb[:, 4:8, :], xt3[:, 4:8, ts(c, TCH)])
                    for k in range(4, KT):
                        nc.sync.dma_start(wg_sb[:, k, :], wg3[:, k, :])
                    nc.gpsimd.dma_start(bg_sb[:], bg_d.ap())
                    nc.gpsimd.dma_start(bu1_sb[:], bu1_d.ap())
                    act_t = apool.tile([P, DT, TCH], F16, name="act_t")
                    act_tiles[c] = act_t
                    glus = [None] * DT
                    for g in range(2):
                        dds = list(range(4 * g, 4 * g + 4))
                        pgs4 = [pgu.tile([P, TCH], F32, name="pg", bufs=4)
                                for _ in dds]
                        for k in range(KT):
                            for i, dd in enumerate(dds):
                                nc.tensor.matmul(
                                    pgs4[i][:], wg_sb[:, k, ts(dd, P)], xt_sb[:, k, :],
                                    start=(k == 0), stop=(k == KT - 1),
                                )
                        if g == 0:
                            # up weights: consumed right after the gate phase
                            nc.sync.dma_start(wu_sb[:], wu3[:])
                        for i, dd in enumerate(dds):
                            glus[dd] = emit_glu(dd, pgs4[i])
                    # down-path constants: consumed by emit_down(0)
                    nc.sync.dma_start(wr_sb[:], wr_d.ap())
                    nc.sync.dma_start(bdb_sb[:], bdb_d.ap())
                    for dd in range(DT):
                        emit_up_act(dd, xt_sb, act_t, glus[dd])
                    nc.sync.dma_start(wd_sb[:], wd3[:])
                else:
                    nc.sync.dma_start(xt_sb[:], xt3[:, :, ts(c, TCH)])
                    emit_gateup(c, xt_sb)
                if c > 0:
                    emit_down(c - 1)
            emit_down(NCH - 1)

    nc.finalize()
    return nc


def make_in_maps(hidden_states, routing_weights, gate_up_proj, gate_up_proj_bias,
                 down_proj, down_proj_bias):
    T = hidden_states.shape[0]
    xt = np.ascontiguousarray(np.asarray(hidden_states, dtype=np.float32).T).astype(np.float16)
    gu = np.asarray(gate_up_proj, dtype=np.float32)
    gub = np.asarray(gate_up_proj_bias, dtype=np.float32)
    wd = np.asarray(down_proj, dtype=np.float32)
    bd = np.asarray(down_proj_bias, dtype=np.float32)
    wr = np.asarray(routing_weights, dtype=np.float32)

    in_maps = []
    for e in range(NUM_EXPERTS):
        in_maps.append({
            "xt": xt,
            "wg": np.ascontiguousarray(gu[e, :, 0::2]).astype(np.float16),
            "wu": np.ascontiguousarray(gu[e, :, 1::2]).astype(np.float16),
            "wd": np.ascontiguousarray(wd[e]).astype(np.float16),
            "bg": np.ascontiguousarray(gub[e, 0::2].reshape(D // P, P).T),
            "bu1": np.ascontiguousarray((gub[e, 1::2] + 1.0).reshape(D // P, P).T),
            "bdb": np.ascontiguousarray(np.broadcast_to(bd[e], (P, H))),
            "wr": np.ascontiguousarray(wr[:, e].reshape(T // P, P).T),
        })
    return in_maps


_NC_CACHE = {}


def _get_nc(T=4096):
    if T not in _NC_CACHE:
        _NC_CACHE[T] = build_nc(T)
    return _NC_CACHE[T]


def run(inputs, trace=False, trace_cores=None, **kwargs):
    """Build (cached), run on 8 cores, return (full_output, BassKernelResults)."""
    T = inputs["hidden_states"].shape[0]
    nc = _get_nc(T)
    in_maps = make_in_maps(**inputs)
    res = run_bass_kernel_spmd(
        nc, in_maps, core_ids=list(range(NUM_EXPERTS)),
        trace=trace, trace_cores=trace_cores, **kwargs,
    )
    out = np.zeros((T, H), np.float32)
    for c in range(NUM_EXPERTS):
        out += res.results[c]["out"]
    return out, res


def kernel(hidden_states, routing_weights, gate_up_proj, gate_up_proj_bias,
           down_proj, down_proj_bias):
    out, _ = run(dict(
        hidden_states=np.asarray(hidden_states),
        routing_weights=np.asarray(routing_weights),
        gate_up_proj=np.asarray(gate_up_proj),
        gate_up_proj_bias=np.asarray(gate_up_proj_bias),
        down_proj=np.asarray(down_proj),
        down_proj_bias=np.asarray(down_proj_bias),
    ))
    return out



# revision 11
# speedup vs baseline: 1.1435x; 1.1435x over previous
"""MoE experts kernel (GPT-OSS style, dense routing over 8 experts) for 8 trn2 NeuronCores.

Strategy: expert-parallel. Core e computes its expert's full MLP for all 4096
tokens, scaled by that expert's routing weight column; the host sums the 8
partial outputs (the unshard step for expert-parallel sharding).

Everything runs in a transposed [feature, token] layout on-chip so that no
transposes are needed anywhere:
  gate   = Wg^T @ X^T          (Wg [H,D] natural = lhsT, X^T [H,T] natural = rhs)
  up     = Wu^T @ X^T
  act    = (up + bu + 1) * gelu_sigmoid(gate + bg)     [D, T] fp16
  out    = (act^T @ Wd + bd) * w_route                 (act tile = lhsT, Wd = rhs)
giving out in [T, H] layout directly. The +bd and *w_route fold into the two
DVE ops that drain the down-projection PSUM.

Gate and down matmuls run in fp16 (4x the mantissa of bf16, same PE speed;
all values here are O(10) so fp16 range is ample). The UP matmul runs in
fp8(e4m3) with perf_mode=DoubleRow (2 k-rows per PE pass): the up-path is the
most quantization-tolerant spot in the network -- its error is damped by the
multiply with glu (~0.3 typical) -- and e4m3 with per-column weight scales
lands at 1.2e-2 max-rel end to end (vs the 2e-2 gate; fp8 anywhere else
fails it). Both fp8 operands are quantized on the host (exact power-of-2
scales; the descale rides the PSUM-drain op's per-partition scale vector).
PSUM accumulation is fp32.
Measured: ~354 us baseline; fp16 MM roofline ~332 us; fp8-up cuts the up
third of the PE stream by ~1.4-1.8x.
"""

import numpy as np
import ml_dtypes

import concourse.mybir as mybir
import concourse.tile as tile
from concourse import bacc
from concourse.bass import ts, ds
from concourse.bass_utils import run_bass_kernel_spmd

AF = mybir.ActivationFunctionType
OP = mybir.AluOpType
F16 = mybir.dt.float16
F32 = mybir.dt.float32
F8 = mybir.dt.float8e4
DR = mybir.MatmulPerfMode.DoubleRow

P = 128
H = 1024          # hidden dim
D = 1024          # expert dim
NUM_EXPERTS = 8

# Gelu_apprx_sigmoid LUT computes x*sigmoid(1.702x) in one ACT pass.
# Fallback (False) builds it from Sigmoid + 2 DVE ops (simulator-friendly).
USE_GELU_LUT = True


def build_nc(T=4096, use_gelu_lut=USE_GELU_LUT):
    KT = H // P            # k-tiles for gate/up matmul (contraction over H)
    KD = D // P            # k-tiles for down matmul (contraction over D)
    DT = D // P            # d-tiles of the expert dim
    TCH = 512              # token chunk = psum free dim
    NCH = T // TCH         # token chunks
    TTILES = TCH // P      # 128-token tiles per chunk
    HCH = 512              # h chunk of the down matmul output
    NHCH = H // HCH

    nc = bacc.Bacc("TRN2", debug=False, enable_asserts=False, num_devices=NUM_EXPERTS)

    xt_d = nc.dram_tensor("xt", [H, T], F16, kind="ExternalInput")
    xt8_d = nc.dram_tensor("xt8", [H, T], F8, kind="ExternalInput")
    wg_d = nc.dram_tensor("wg", [H, D], F16, kind="ExternalInput")
    wu8_d = nc.dram_tensor("wu8", [H, D], F8, kind="ExternalInput")
    wd_d = nc.dram_tensor("wd", [D, H], F16, kind="ExternalInput")
    cvec_d = nc.dram_tensor("cvec", [P, DT], F32, kind="ExternalInput")
    bg_d = nc.dram_tensor("bg", [P, DT], F32, kind="ExternalInput")
    bu1_d = nc.dram_tensor("bu1", [P, DT], F32, kind="ExternalInput")
    bdb_d = nc.dram_tensor("bdb", [P, H], F32, kind="ExternalInput")
    wr_d = nc.dram_tensor("wr", [P, T // P], F32, kind="ExternalInput")
    out_d = nc.dram_tensor("out", [T, H], F32, kind="ExternalOutput")
    out_ap = out_d.ap()

    with tile.TileContext(nc) as tc:
        with (
            tc.tile_pool(name="wpool", bufs=1) as wpool,
            tc.tile_pool(name="xpool", bufs=3) as xpool,
            tc.tile_pool(name="gpool", bufs=3) as gpool,
            tc.tile_pool(name="apool", bufs=2) as apool,
            tc.tile_pool(name="opool", bufs=4) as opool,
            tc.tile_pool(name="pgu", bufs=2, space="PSUM") as pgu,
            tc.tile_pool(name="pdn", bufs=3, space="PSUM") as pdn,
        ):
            # --- resident weights/biases ---
            # DMA *issue* costs ~0.6us each on the sync engine's queue, so
            # the emission order below is chosen to put the first matmul's
            # dependencies at the very front of the queue, and everything
            # else behind the point where it is first consumed.
            bg_sb = wpool.tile([P, DT], F32, name="bg_sb")
            bu1_sb = wpool.tile([P, DT], F32, name="bu1_sb")
            cvec_sb = wpool.tile([P, DT], F32, name="cvec_sb")
            wr_sb = wpool.tile([P, T // P], F32, name="wr_sb")
            bdb_sb = wpool.tile([P, H], F32, name="bdb_sb")

            wg_sb = wpool.tile([P, KT, D], F16, name="wg_sb")
            wu8_sb = wpool.tile([P, KT, D], F8, name="wu8_sb")
            wd_sb = wpool.tile([P, KD, H], F16, name="wd_sb")
            wg3 = wg_d.ap().rearrange("(ko p) d -> p ko d", p=P)
            wu3 = wu8_d.ap().rearrange("(ko p) d -> p ko d", p=P)
            wd3 = wd_d.ap().rearrange("(ko p) h -> p ko h", p=P)

            act_tiles = [None] * NCH
            GLU_BUFS = DT + 2

            def emit_gate_mms(dd, xt_sb):
                pg = pgu.tile([P, TCH], F32, name="pg", bufs=4)
                for k in range(KT):
                    nc.tensor.matmul(
                        pg[:], wg_sb[:, k, ts(dd, P)], xt_sb[:, k, :],
                        start=(k == 0), stop=(k == KT - 1),
                    )
                return pg

            def emit_glu(dd, pg):
                glu_t = gpool.tile([P, TCH], F16, name="glu_t", bufs=GLU_BUFS)
                if use_gelu_lut:
                    # glu = g*sigmoid(1.702 g), g = psum_gate + bg
                    nc.scalar.activation(
                        glu_t[:], pg[:], AF.Gelu_apprx_sigmoid,
                        bias=bg_sb[:, dd:dd + 1], scale=1.0,
                    )
                else:
                    g_t = gpool.tile([P, TCH], F32, name="g_t")
                    nc.vector.tensor_scalar(
                        g_t[:], pg[:], bg_sb[:, dd:dd + 1], None, OP.add,
                    )
                    s_t = gpool.tile([P, TCH], F16, name="s_t")
                    nc.scalar.activation(s_t[:], g_t[:], AF.Sigmoid, scale=1.702)
                    nc.vector.tensor_mul(glu_t[:], g_t[:], s_t[:])
                return glu_t

            def emit_up_act(dd, xt8_sb, act_t, glu_t):
                pu = pgu.tile([P, TCH], F32, name="pu", bufs=2)
                # fp8 DoubleRow: each matmul consumes 2 k-tiles (the PE holds
                # 2 fp8 weights per cell and double-pumps the contraction).
                for k2 in range(KT // 2):
                    nc.tensor.matmul(
                        pu[:], wu8_sb[:, 2 * k2:2 * k2 + 2, ts(dd, P)],
                        xt8_sb[:, 2 * k2:2 * k2 + 2, :],
                        start=(k2 == 0), stop=(k2 == KT // 2 - 1),
                        perf_mode=DR,
                    )
                # up+1 = psum*(1/(su*sx)) + (bu+1): per-partition descale and
                # bias ride the scalar engine; DVE then multiplies by glu.
                up1_t = gpool.tile([P, TCH], F16, name="up1_t")
                nc.scalar.activation(
                    up1_t[:], pu[:], AF.Identity,
                    bias=bu1_sb[:, dd:dd + 1], scale=cvec_sb[:, dd:dd + 1],
                )
                nc.vector.tensor_mul(act_t[:, dd, :], up1_t[:], glu_t[:])

            def emit_gateup(c, xt_sb, xt8_sb):
                act_t = apool.tile([P, DT, TCH], F16, name="act_t")
                act_tiles[c] = act_t
                for dd in range(DT):
                    pg = emit_gate_mms(dd, xt_sb)
                    glu_t = emit_glu(dd, pg)
                    emit_up_act(dd, xt8_sb, act_t, glu_t)

            def emit_down(c):
                act_t = act_tiles[c]
                for tt in range(TTILES):
                    tcol = c * TTILES + tt
                    # both h-halves drain into one [P, H] tile -> one out DMA
                    ot = opool.tile([P, H], F32, name="ot", bufs=2)
                    for hh in range(NHCH):
                        po = pdn.tile([P, HCH], F32, name="po", bufs=2)
                        for kd in range(KD):
                            nc.tensor.matmul(
                                po[:], act_t[:, kd, ts(tt, P)], wd_sb[:, kd, ts(hh, HCH)],
                                start=(kd == 0), stop=(kd == KD - 1),
                            )
                        # out = (psum + bd) * w_route[t]
                        qt = opool.tile([P, HCH], F32, name="qt")
                        nc.vector.tensor_add(qt[:], po[:], bdb_sb[:, ts(hh, HCH)])
                        nc.vector.tensor_scalar(
                            ot[:, ts(hh, HCH)], qt[:], wr_sb[:, tcol:tcol + 1], None,
                            OP.mult,
                        )
                    # issue from the (otherwise idle) gpsimd queue: a
                    # different DMA ring than the input loads, so tail
                    # transfers overlap.
                    nc.gpsimd.dma_start(
                        out_ap[ds(c * TCH + tt * P, P), :], ot[:],
                    )

            xt3 = xt_d.ap().rearrange("(ko p) t -> p ko t", p=P)
            xt83 = xt8_d.ap().rearrange("(ko p) t -> p ko t", p=P)
            for c in range(NCH):
                xt_sb = xpool.tile([P, KT, TCH], F16, name="xt_sb")
                xt8_sb = xpool.tile([P, KT, TCH], F8, name="xt8_sb")
                if c == 0:
                    # PE warm-up: matmuls on a zeroed tile, dependent only on
                    # a local memset, stream from ~6.5us (right after the NEFF
                    # prolog) while the first weight DMA is still in flight.
                    # They carry the PE through its p-state ramp so the first
                    # real matmuls run at full clock, and they keep the PE
                    # busy during the first (cold, slow) HBM transfer.
                    warm_sb = wpool.tile([P, TCH], F16, name="warm_sb")
                    nc.gpsimd.memset(warm_sb[:], 0.0)
                    for w in range(8):
                        pwu = pgu.tile([P, TCH], F32, name="pg", bufs=4)
                        nc.tensor.matmul(
                            pwu[:], warm_sb[:, :P], warm_sb[:], start=True, stop=True,
                        )
                    # Startup choreography. The matmul stream becomes dense as
                    # soon as the first k-slices land: the gate phase runs
                    # k-outer over dd-groups of 4 (4 psum banks), so each
                    # arriving (wg_k, xt) slice immediately feeds 4 matmuls.
                    # The first wg slice is split in half (dds 0-3 only) so
                    # the first real matmul's dependency is a 128KB transfer;
                    # xt rides the scalar queue in parallel; biases ride
                    # gpsimd.
                    nc.sync.dma_start(wg_sb[:, 0, :512], wg3[:, 0, :512])
                    nc.scalar.dma_start(xt_sb[:, 0, :], xt3[:, 0, ts(c, TCH)])
                    nc.sync.dma_start(wg_sb[:, 0, 512:], wg3[:, 0, 512:])
                    nc.scalar.dma_start(xt_sb[:, 1:4, :], xt3[:, 1:4, ts(c, TCH)])
                    nc.sync.dma_start(wg_sb[:, 1, :], wg3[:, 1, :])
                    nc.sync.dma_start(wg_sb[:, 2, :], wg3[:, 2, :])
                    nc.sync.dma_start(wg_sb[:, 3, :], wg3[:, 3, :])
                    nc.scalar.dma_start(xt_sb[:, 4:8, :], xt3[:, 4:8, ts(c, TCH)])
                    for k in range(4, KT):
                        nc.sync.dma_start(wg_sb[:, k, :], wg3[:, k, :])
                    nc.gpsimd.dma_start(bg_sb[:], bg_d.ap())
                    nc.gpsimd.dma_start(bu1_sb[:], bu1_d.ap())
                    act_t = apool.tile([P, DT, TCH], F16, name="act_t")
                    act_tiles[c] = act_t
                    glus = [None] * DT
                    for g in range(2):
                        dds = list(range(4 * g, 4 * g + 4))
                        pgs4 = [pgu.tile([P, TCH], F32, name="pg", bufs=4)
                                for _ in dds]
                        for k in range(KT):
                            for i, dd in enumerate(dds):
                                nc.tensor.matmul(
                                    pgs4[i][:], wg_sb[:, k, ts(dd, P)], xt_sb[:, k, :],
                                    start=(k == 0), stop=(k == KT - 1),
                                )
                        if g == 0:
                            # up weights + fp8 tokens: consumed right after
                            # the gate phase
                            nc.sync.dma_start(wu8_sb[:], wu3[:])
                            nc.scalar.dma_start(
                                xt8_sb[:], xt83[:, :, ts(c, TCH)])
                            nc.gpsimd.dma_start(cvec_sb[:], cvec_d.ap())
                        for i, dd in enumerate(dds):
                            glus[dd] = emit_glu(dd, pgs4[i])
                    # down-path constants: consumed by emit_down(0)
                    nc.sync.dma_start(wr_sb[:], wr_d.ap())
                    nc.sync.dma_start(bdb_sb[:], bdb_d.ap())
                    for dd in range(DT):
                        emit_up_act(dd, xt8_sb, act_t, glus[dd])
                    nc.sync.dma_start(wd_sb[:], wd3[:])
                else:
                    nc.sync.dma_start(xt_sb[:], xt3[:, :, ts(c, TCH)])
                    nc.scalar.dma_start(xt8_sb[:], xt83[:, :, ts(c, TCH)])
                    emit_gateup(c, xt_sb, xt8_sb)
                if c > 0:
                    emit_down(c - 1)
            emit_down(NCH - 1)

    nc.finalize()
    return nc


def _q8(x, scale):
    """quantize x*scale to TRN e4m3 (max normal 240), round-to-nearest."""
    return np.clip(x * scale, -240.0, 240.0).astype(ml_dtypes.float8_e4m3)


def make_in_maps(hidden_states, routing_weights, gate_up_proj, gate_up_proj_bias,
                 down_proj, down_proj_bias):
    T = hidden_states.shape[0]
    x32 = np.ascontiguousarray(np.asarray(hidden_states, dtype=np.float32).T)
    xt = x32.astype(np.float16)
    gu = np.asarray(gate_up_proj, dtype=np.float32)
    gub = np.asarray(gate_up_proj_bias, dtype=np.float32)
    wd = np.asarray(down_proj, dtype=np.float32)
    bd = np.asarray(down_proj_bias, dtype=np.float32)
    wr = np.asarray(routing_weights, dtype=np.float32)

    # fp8 tokens: one global power-of-2 scale (exact to descale)
    sx = 2.0 ** np.floor(np.log2(224.0 / max(np.abs(x32).max(), 1e-12)))
    xt8 = _q8(x32, sx)

    in_maps = []
    for e in range(NUM_EXPERTS):
        wu = np.ascontiguousarray(gu[e, :, 1::2])           # [H, D]
        # per-output-column power-of-2 scales for the up weights
        amax = np.maximum(np.abs(wu).max(axis=0), 1e-12)
        su = 2.0 ** np.floor(np.log2(224.0 / amax))         # [D]
        cvec = (1.0 / (su * sx)).astype(np.float32)
        in_maps.append({
            "xt": xt,
            "xt8": xt8,
            "wg": np.ascontiguousarray(gu[e, :, 0::2]).astype(np.float16),
            "wu8": _q8(wu, su[None, :]),
            "wd": np.ascontiguousarray(wd[e]).astype(np.float16),
            "cvec": np.ascontiguousarray(cvec.reshape(D // P, P).T),
            "bg": np.ascontiguousarray(gub[e, 0::2].reshape(D // P, P).T),
            "bu1": np.ascontiguousarray((gub[e, 1::2] + 1.0).reshape(D // P, P).T),
            "bdb": np.ascontiguousarray(np.broadcast_to(bd[e], (P, H))),
            "wr": np.ascontiguousarray(wr[:, e].reshape(T // P, P).T),
        })
    return in_maps


_NC_CACHE = {}


def _get_nc(T=4096):
    if T not in _NC_CACHE:
        _NC_CACHE[T] = build_nc(T)
    return _NC_CACHE[T]


def run(inputs, trace=False, trace_cores=None, **kwargs):
    """Build (cached), run on 8 cores, return (full_output, BassKernelResults)."""
    T = inputs["hidden_states"].shape[0]
    nc = _get_nc(T)
    in_maps = make_in_maps(**inputs)
    res = run_bass_kernel_spmd(
        nc, in_maps, core_ids=list(range(NUM_EXPERTS)),
        trace=trace, trace_cores=trace_cores, **kwargs,
    )
    out = np.zeros((T, H), np.float32)
    for c in range(NUM_EXPERTS):
        out += res.results[c]["out"]
    return out, res


def kernel(hidden_states, routing_weights, gate_up_proj, gate_up_proj_bias,
           down_proj, down_proj_bias):
    out, _ = run(dict(
        hidden_states=np.asarray(hidden_states),
        routing_weights=np.asarray(routing_weights),
        gate_up_proj=np.asarray(gate_up_proj),
        gate_up_proj_bias=np.asarray(gate_up_proj_bias),
        down_proj=np.asarray(down_proj),
        down_proj_bias=np.asarray(down_proj_bias),
    ))
    return out

